# revision 40
# baseline (speedup 1.0000x reference)
"""AttentionPooling (segment softmax-pool) Trainium2 kernel — v2.

Graphs are sharded across 8 cores (1024 graphs each); nodes follow their graph
(batch is sorted). Each core's graphs form 8 windows of 128 graphs; a window's
nodes are host-padded to a fixed count T and processed in groups of 512.

out[g] = (sum_{n in g} e_n * x_n) / (sum_{n in g} e_n + 1e-8), with
e_n = exp(tanh(x_n @ W1 + b1) @ W2 + b2); the division is pulled out of the
node loop so one pass over x suffices.

v2 changes vs v1:
  - attention-path x (xt, transposed layout) and W1 ship in fp8e4; mm1 runs
    as 2 DoubleRow matmuls per group (virtual K=256, 2x PE rate, half the
    xt HBM bytes).  Value-path x (xn) stays bf16 (fp8 there fails the 2e-2
    error gate); measured end-to-end rel err ~1.3e-2.
  - exp is batched per window: mm2 logits accumulate into a window-level
    [128, 64] PSUM tile, one Exp ACT op per window (vs 128 tiny ones).
  - window-lag software pipeline: during window w's mm1/tanh/mm2, PE also
    runs window w-1's seg matmuls (S built by DVE from the already-computed
    e's).  PSUM budget: 2x2-bank ph + 2x1-bank pl + 2x1-bank pseg = 8 banks.
  - 3-deep window buffering so DMA prefetch runs 2 windows ahead.

Per 512-node group:
  mm1:  h^T[hid_out, node] = W1_dr.T @ x^T_dr   (fp8 DoubleRow, 2 matmuls)
  tanh: one ACT op PSUM->SBUF bf16 (fused [128,1024] if b1==0)
  mm2:  pl_w[:, g*4+t] += ht_chunk.T @ W2_chunk  (bf16, k-accumulated)
  (window end) exp: one ACT op on [128, 64] logits (+b2)
  S[node, graph] = (iota == batch_rel) * e    (one fused DVE tensor_scalar)
  seg:  psum[graph, 0:257] += S.T @ [x | 1]   (accumulated over the window)
Window end: out = psum[:,0:256] / (psum[:,256] + eps) -> one DMA.
"""
import os
import sys

for _p in ("/opt/trn_rl_repo", "/root/.axon_site/_ro/trn_rl_repo"):
    if os.path.isdir(_p) and _p not in sys.path:
        sys.path.insert(0, _p)

import numpy as np
import ml_dtypes

import concourse.bacc as bacc
import concourse.tile as tile
from concourse import mybir
from concourse.bass_utils import run_bass_kernel_spmd

F32 = mybir.dt.float32
BF16 = mybir.dt.bfloat16
FP8 = mybir.dt.float8e4
BF = ml_dtypes.bfloat16
F8 = ml_dtypes.float8_e4m3

N_GRAPHS = 8192
HIDDEN = 256
CORES = 8
WPC = 8            # windows per core
WG = 128           # graphs per window
GRP = 512          # nodes per group
ROW = 258          # xn row: 256 x + 1.0 + pad
EPS = 1e-8

# bf16 const block: W2 chunk k at col k; iota row at 2:130
C_W2 = 0
C_IOTA = 2
CBW = 130


def _build_program(T: int, variant: str = "full", zero_bias: bool = False):
    ng = T // GRP
    cols = T // 128
    XNW = ng * 4 * ROW
    XTW = ng * 1024

    nc = bacc.Bacc("TRN2", target_bir_lowering=False, debug=False,
                   num_devices=CORES)
    xn = nc.dram_tensor("xn", [WPC, 128, XNW], BF16, kind="ExternalInput").ap()
    xt = nc.dram_tensor("xt", [WPC, 128, ng, 2, GRP], FP8,
                        kind="ExternalInput").ap()
    br = nc.dram_tensor("br", [WPC, 128, cols], F32, kind="ExternalInput").ap()
    cbf = nc.dram_tensor("cbf", [128, CBW], BF16, kind="ExternalInput").ap()
    cw1 = nc.dram_tensor("cw1", [128, 2, 2, 128], FP8,
                         kind="ExternalInput").ap()
    cf = nc.dram_tensor("cf", [128, 131], F32, kind="ExternalInput").ap()
    out = nc.dram_tensor("out", [WPC * WG, HIDDEN], F32, kind="ExternalOutput").ap()

    from contextlib import ExitStack
    with tile.TileContext(nc) as tc:
        with ExitStack() as ctx:
            cpool = ctx.enter_context(tc.tile_pool(name="const", bufs=1))
            brpool = ctx.enter_context(tc.tile_pool(name="brp", bufs=3))
            xnpool = ctx.enter_context(tc.tile_pool(name="xnp", bufs=3))
            xtpool = ctx.enter_context(tc.tile_pool(name="xtp", bufs=3))
            htpool = ctx.enter_context(tc.tile_pool(name="htp", bufs=4))
            etpool = ctx.enter_context(tc.tile_pool(name="etp", bufs=12))
            spool = ctx.enter_context(tc.tile_pool(name="sp", bufs=28))
            owpool = ctx.enter_context(tc.tile_pool(name="ow", bufs=3))
            phpool = ctx.enter_context(tc.tile_pool(name="ph", bufs=3, space="PSUM"))
            pspool = ctx.enter_context(tc.tile_pool(name="ps", bufs=2, space="PSUM"))

            cb = cpool.tile([128, CBW], BF16)
            cw = cpool.tile([128, 2, 2, 128], FP8)
            cft = cpool.tile([128, 131], F32)
            nc.sync.dma_start(out=cw[:], in_=cw1[:])
            nc.sync.dma_start(out=cb[:], in_=cbf[:])
            nc.sync.dma_start(out=cft[:], in_=cf[:])
            iota = cb[:, C_IOTA:C_IOTA + 128]

            wstate = {}

            def load_pass1(w, first=False):
                # xt (chunked, issued first) + br: feeds mm1/mm2 of window w.
                # Issued on the sync HWDGE queue.  The first window uses
                # small leading chunks so mm1 can start right after the
                # framework preamble.
                if w >= WPC:
                    return
                xtwt = xtpool.tile([128, ng, 2, GRP], FP8)
                sizes = [4, 4, 8] if first else [8, 8]
                q = 0
                for qn in sizes:
                    qn = min(qn, ng - q)
                    if qn <= 0:
                        break
                    nc.sync.dma_start(out=xtwt[:, q:q + qn],
                                      in_=xt[w, :, q:q + qn])
                    q += qn
                brw = brpool.tile([128, cols], F32)
                nc.sync.dma_start(out=brw[:], in_=br[w])
                wstate[w] = dict(brw=brw, xtwt=xtwt)

            def load_pass2(w):
                # xn (chunked): feeds seg of window w (runs during iter w+1)
                if w >= WPC:
                    return
                xnwt = xnpool.tile([128, XNW], BF16)
                cq = XNW // 2
                for q in range(2):
                    nc.sync.dma_start(out=xnwt[:, q * cq:(q + 1) * cq],
                                      in_=xn[w, :, q * cq:(q + 1) * cq])
                wstate[w]["xnwt"] = xnwt

            def emit_mm1_tanh(w, g):
                ws = wstate[w]
                ph = phpool.tile([128, 2, GRP], F32)
                xt3 = ws["xtwt"][:, g]
                for m in range(2):
                    nc.tensor.matmul(ph[:, m, :], cw[:, m], xt3,
                                     start=True, stop=True,
                                     perf_mode=mybir.MatmulPerfMode.DoubleRow)
                ht = htpool.tile([128, 2, GRP], BF16)
                if zero_bias:
                    nc.scalar.activation(ht[:, :, :], ph[:, :, :],
                                         mybir.ActivationFunctionType.Tanh,
                                         bias=0.0, scale=1.0)
                else:
                    for m in range(2):
                        nc.scalar.activation(ht[:, m, :], ph[:, m, :],
                                             mybir.ActivationFunctionType.Tanh,
                                             bias=cft[:, m:m + 1], scale=1.0)
                ws.setdefault("ht", {})[g] = ht

            def emit_mm2(w, g):
                # plw shares a PSUM bank with pseg: exactly ONE start=True
                # (the bank's first matmul) pending-zeroes the whole 2KB
                # bank; every later matmul relies on per-element has_written
                # (first write overwrites, second accumulates).
                ws = wstate[w]
                if g == 0:
                    pcomb = pspool.tile([128, 324], F32, name="pcomb")
                    ws["pcomb"] = pcomb
                    ws["plw"] = pcomb[:, 260:324]
                    ws["pseg"] = pcomb[:, 0:257]
                ht = ws["ht"].pop(g)
                plw = ws["plw"]
                for t in range(4):
                    c = g * 4 + t
                    for k in range(2):
                        nc.tensor.matmul(plw[:, c:c + 1],
                                         ht[:, k, 128 * t:128 * (t + 1)],
                                         cb[:, C_W2 + k:C_W2 + k + 1],
                                         start=(g == 0 and t == 0 and k == 0),
                                         stop=False, skip_group_check=True)

            NQ = 2  # exp slices per window

            def emit_exp(w, q):
                # exp on quarter-window q's logit columns
                ws = wstate[w]
                qc = cols // NQ
                et = etpool.tile([128, qc], F32)
                ebias = 0.0 if zero_bias else cft[:, 2:3]
                nc.scalar.activation(et[:], ws["plw"][:, q * qc:(q + 1) * qc],
                                     mybir.ActivationFunctionType.Exp,
                                     bias=ebias, scale=1.0)
                ws.setdefault("et", {})[q] = et

            def emit_sbuild(w, g):
                ws = wstate[w]
                qc = cols // NQ
                q = (g * 4) // qc
                et = ws["et"][q]
                sts = []
                for t in range(4):
                    c = g * 4 + t
                    st = spool.tile([128, 128], BF16)
                    nc.vector.tensor_scalar(st[:], iota,
                                            ws["brw"][:, c:c + 1],
                                            et[:, c - q * qc:c - q * qc + 1],
                                            op0=mybir.AluOpType.is_equal,
                                            op1=mybir.AluOpType.mult)
                    sts.append(st)
                ws.setdefault("sts", {})[g] = sts

            def emit_seg(w, g):
                ws = wstate[w]
                sts = ws["sts"].pop(g)
                for t in range(4):
                    c = g * 4 + t
                    nc.tensor.matmul(ws["pseg"][:],
                                     sts[t][:],
                                     ws["xnwt"][:, c * ROW:c * ROW + 257],
                                     start=False,
                                     stop=(g == ng - 1 and t == 3),
                                     skip_group_check=True)
                if g == ng - 1:
                    finalize_window(w)

            def finalize_window(w):
                ws = wstate.pop(w)
                pseg = ws["pseg"]
                dtmp = owpool.tile([128, 1], F32)
                nc.vector.tensor_scalar_add(dtmp[:], pseg[:, 256:257], EPS)
                rec = owpool.tile([128, 1], F32)
                nc.vector.reciprocal(rec[:], dtmp[:])
                ow = owpool.tile([128, HIDDEN], F32)
                nc.vector.tensor_scalar(ow[:], pseg[:, 0:256], rec[:], None,
                                        op0=mybir.AluOpType.mult)
                nc.sync.dma_start(out=out[w * WG:(w + 1) * WG, :], in_=ow[:])

            # ACT warmup: force the exp/tanh table load at t=0 so the first
            # real tanh doesn't pay the ~2.7us ACT_TABLE_LOAD.
            wz = owpool.tile([128, 1], F32)
            nc.vector.memset(wz[:], 0.0)
            wu = owpool.tile([128, 1], F32)
            nc.scalar.activation(wu[:], wz[:],
                                 mybir.ActivationFunctionType.Exp,
                                 bias=0.0, scale=1.0)
            # PE warmup: keep the PE busy while the first window loads so
            # the HAM clock-gate reaches 8/8 before real compute begins.
            dmy = owpool.tile([128, 257], BF16)
            nc.vector.memset(dmy[:], 0.0)
            pdmy = pspool.tile([128, 324], F32, name="pcomb")
            for _ in range(24):
                nc.tensor.matmul(pdmy[:, 0:257], dmy[:, 0:128], dmy[:, 0:257],
                                 start=True, stop=True)

            # main pipeline over flat (window, group) steps; seg lags
            # mm1/mm2 by half a window (exp runs per quarter-window) and
            # S-builds lead their seg by LEAD steps so DVE is never on the
            # PE's critical path.
            steps = [(w, g) for w in range(WPC) for g in range(ng)]
            LAG = 12
            LEAD = 4
            n = len(steps)
            load_pass1(0, first=True)
            load_pass1(1)
            load_pass2(0)
            for i in range(n + LAG):
                if i < n:
                    w, g = steps[i]
                    emit_mm1_tanh(w, g)
                    emit_mm2(w, g)
                    if (g + 1) % (ng // NQ) == 0:
                        emit_exp(w, (g * NQ) // ng)
                j = i - (LAG - LEAD)
                if 0 <= j < n:
                    emit_sbuild(*steps[j])
                if i >= LAG:
                    emit_seg(*steps[i - LAG])
                if i < n:
                    w, g = steps[i]
                    if g == 2:
                        load_pass1(w + 2)
                    elif g == 6:
                        load_pass2(w + 1)
    nc.compile()
    return nc


def _prep_inputs(x, batch, W1, b1, W2, b2):
    batch = np.asarray(batch).astype(np.int64)
    x = np.asarray(x, dtype=np.float32)

    bnds = np.searchsorted(batch, np.arange(0, N_GRAPHS + 1, WG))
    sizes = np.diff(bnds)
    T = int(max(512, ((int(sizes.max()) + GRP - 1) // GRP) * GRP))
    ng = T // GRP
    cols = T // 128

    xbf = x.astype(BF)
    x8 = x.astype(F8)
    batf = batch.astype(np.float32)

    W1 = np.asarray(W1, np.float32)
    cbf = np.zeros((128, CBW), dtype=BF)
    for k in range(2):
        cbf[:, C_W2 + k] = np.asarray(W2, np.float32)[128 * k:128 * (k + 1), 0].astype(BF)
    cbf[:, C_IOTA:C_IOTA + 128] = np.tile(
        np.arange(128, dtype=np.float32), (128, 1)).astype(BF)

    # W1 DoubleRow: cw1[p, mb, i, m] = W1[p + 128*i, 128*mb + m]
    cw1 = np.zeros((128, 2, 2, 128), dtype=F8)
    for mb in range(2):
        for i in range(2):
            cw1[:, mb, i, :] = \
                W1[128 * i:128 * (i + 1), 128 * mb:128 * (mb + 1)].astype(F8)

    cf = np.zeros((128, 131), dtype=np.float32)
    cf[:, 3:131] = np.tile(np.arange(128, dtype=np.float32), (128, 1))
    cf[:, 0] = np.asarray(b1, np.float32)[0:128]
    cf[:, 1] = np.asarray(b1, np.float32)[128:256]
    cf[:, 2] = float(np.asarray(b2, np.float32).reshape(-1)[0])
    zero_bias = bool((np.asarray(b1) == 0).all() and (np.asarray(b2) == 0).all())

    in_maps = []
    for c in range(CORES):
        xn_c = np.zeros((WPC, T, ROW), dtype=BF)
        xt_lin = np.zeros((WPC, 2, 128, T), dtype=F8)
        br_c = np.full((WPC, 128, cols), -1.0, dtype=np.float32)
        for w in range(WPC):
            j = c * WPC + w
            s, e = int(bnds[j]), int(bnds[j + 1])
            sz = e - s
            if sz:
                xn_c[w, :sz, 0:256] = xbf[s:e]
                xn_c[w, :sz, 256] = BF(1.0)
                xt_lin[w, 0, :, :sz] = x8[s:e, 0:128].T
                xt_lin[w, 1, :, :sz] = x8[s:e, 128:256].T
                tmp = np.full(T, -1.0, dtype=np.float32)
                tmp[:sz] = batf[s:e] - (c * 1024 + w * WG)
                br_c[w] = tmp.reshape(cols, 128).T
        # xn swizzle: [w, g*512+t*128+p, d] -> [w, p, (g*4+t)*ROW + d]
        xn_sw = np.ascontiguousarray(
            xn_c.reshape(WPC, ng, 4, 128, ROW).transpose(0, 3, 1, 2, 4)
        ).reshape(WPC, 128, ng * 4 * ROW)
        # xt swizzle: [w, c2, p, g*512+n] -> [w, p, g, c2, n]
        xt_sw = np.ascontiguousarray(
            xt_lin.reshape(WPC, 2, 128, ng, 512).transpose(0, 2, 3, 1, 4))
        in_maps.append(dict(xn=xn_sw, xt=xt_sw, br=br_c, cbf=cbf, cw1=cw1, cf=cf))
    return T, in_maps, zero_bias


_PROGRAM_CACHE = {}


def kernel(x, batch, W1, b1, W2, b2):
    T, in_maps, zb = _prep_inputs(x, batch, W1, b1, W2, b2)
    key = (T, zb)
    if key not in _PROGRAM_CACHE:
        _PROGRAM_CACHE[key] = _build_program(T, zero_bias=zb)
    nc = _PROGRAM_CACHE[key]
    res = run_bass_kernel_spmd(nc, in_maps, list(range(CORES))).results
    return np.concatenate([res[c]["out"] for c in range(CORES)], axis=0)


# revision 41
# speedup vs baseline: 1.1489x; 1.1489x over previous
"""AttentionPooling (segment softmax-pool) Trainium2 kernel — v2.

Graphs are sharded across 8 cores (1024 graphs each); nodes follow their graph
(batch is sorted). Each core's graphs form 8 windows of 128 graphs; a window's
nodes are host-padded to a fixed count T and processed in groups of 512.

out[g] = (sum_{n in g} e_n * x_n) / (sum_{n in g} e_n + 1e-8), with
e_n = exp(tanh(x_n @ W1 + b1) @ W2 + b2); the division is pulled out of the
node loop so one pass over x suffices.

v2 changes vs v1:
  - attention-path x (xt, transposed layout) and W1 ship in fp8e4; mm1 runs
    as 2 DoubleRow matmuls per group (virtual K=256, 2x PE rate, half the
    xt HBM bytes).  Value-path x (xn) stays bf16 (fp8 there fails the 2e-2
    error gate); measured end-to-end rel err ~1.3e-2.
  - exp is batched per window: mm2 logits accumulate into a window-level
    [128, 64] PSUM tile, one Exp ACT op per window (vs 128 tiny ones).
  - window-lag software pipeline: during window w's mm1/tanh/mm2, PE also
    runs window w-1's seg matmuls (S built by DVE from the already-computed
    e's).  PSUM budget: 2x2-bank ph + 2x1-bank pl + 2x1-bank pseg = 8 banks.
  - 3-deep window buffering so DMA prefetch runs 2 windows ahead.

Per 512-node group:
  mm1:  h^T[hid_out, node] = W1_dr.T @ x^T_dr   (fp8 DoubleRow, 2 matmuls)
  tanh: one ACT op PSUM->SBUF bf16 (fused [128,1024] if b1==0)
  mm2:  pl_w[:, g*4+t] += ht_chunk.T @ W2_chunk  (bf16, k-accumulated)
  (window end) exp: one ACT op on [128, 64] logits (+b2)
  S[node, graph] = (iota == batch_rel) * e    (one fused DVE tensor_scalar)
  seg:  psum[graph, 0:257] += S.T @ [x | 1]   (accumulated over the window)
Window end: out = psum[:,0:256] / (psum[:,256] + eps) -> one DMA.
"""
import os
import sys

for _p in ("/opt/trn_rl_repo", "/root/.axon_site/_ro/trn_rl_repo"):
    if os.path.isdir(_p) and _p not in sys.path:
        sys.path.insert(0, _p)

import numpy as np
import ml_dtypes

import concourse.bacc as bacc
import concourse.tile as tile
from concourse import mybir
from concourse.bass_utils import run_bass_kernel_spmd

F32 = mybir.dt.float32
BF16 = mybir.dt.bfloat16
FP8 = mybir.dt.float8e4
BF = ml_dtypes.bfloat16
F8 = ml_dtypes.float8_e4m3

N_GRAPHS = 8192
HIDDEN = 256
CORES = 8
WPC = 8            # windows per core
WG = 128           # graphs per window
GRP = 512          # nodes per group
ROW = 258          # xn row: 256 x + 1.0 + pad
EPS = 1e-8

# bf16 const block: W2 chunk k at col k; iota row at 2:130
C_W2 = 0
C_IOTA = 2
CBW = 130


def _build_program(T: int, variant: str = "full", zero_bias: bool = False):
    ng = T // GRP
    cols = T // 128
    XNW = ng * 4 * ROW
    XTW = ng * 1024

    nc = bacc.Bacc("TRN2", target_bir_lowering=False, debug=False,
                   num_devices=CORES)
    xn = nc.dram_tensor("xn", [WPC, 128, XNW], BF16, kind="ExternalInput").ap()
    xt = nc.dram_tensor("xt", [WPC, 128, ng, 2, GRP], FP8,
                        kind="ExternalInput").ap()
    br = nc.dram_tensor("br", [WPC, 128, cols], F32, kind="ExternalInput").ap()
    cbf = nc.dram_tensor("cbf", [128, CBW], BF16, kind="ExternalInput").ap()
    cw1 = nc.dram_tensor("cw1", [128, 2, 2, 128], FP8,
                         kind="ExternalInput").ap()
    cf = nc.dram_tensor("cf", [128, 131], F32, kind="ExternalInput").ap()
    out = nc.dram_tensor("out", [WPC * WG, HIDDEN], F32, kind="ExternalOutput").ap()

    from contextlib import ExitStack
    with tile.TileContext(nc) as tc:
        with ExitStack() as ctx:
            cpool = ctx.enter_context(tc.tile_pool(name="const", bufs=1))
            brpool = ctx.enter_context(tc.tile_pool(name="brp", bufs=3))
            xnpool = ctx.enter_context(tc.tile_pool(name="xnp", bufs=3))
            xtpool = ctx.enter_context(tc.tile_pool(name="xtp", bufs=3))
            htpool = ctx.enter_context(tc.tile_pool(name="htp", bufs=4))
            etpool = ctx.enter_context(tc.tile_pool(name="etp", bufs=12))
            spool = ctx.enter_context(tc.tile_pool(name="sp", bufs=28))
            owpool = ctx.enter_context(tc.tile_pool(name="ow", bufs=3))
            phpool = ctx.enter_context(tc.tile_pool(name="ph", bufs=3, space="PSUM"))
            pspool = ctx.enter_context(tc.tile_pool(name="ps", bufs=2, space="PSUM"))

            cb = cpool.tile([128, CBW], BF16)
            cw = cpool.tile([128, 2, 2, 128], FP8)
            cft = cpool.tile([128, 131], F32)
            nc.sync.dma_start(out=cw[:], in_=cw1[:])
            nc.sync.dma_start(out=cb[:], in_=cbf[:])
            nc.sync.dma_start(out=cft[:], in_=cf[:])
            iota = cb[:, C_IOTA:C_IOTA + 128]

            wstate = {}

            def load_pass1(w, first=False):
                # xt (chunked, issued first) + br: feeds mm1/mm2 of window w.
                # Issued on the sync HWDGE queue.  The first window uses
                # small leading chunks so mm1 can start right after the
                # framework preamble.
                if w >= WPC:
                    return
                xtwt = xtpool.tile([128, ng, 2, GRP], FP8)
                sizes = [4, 4, 8] if first else [8, 8]
                q = 0
                for qn in sizes:
                    qn = min(qn, ng - q)
                    if qn <= 0:
                        break
                    nc.sync.dma_start(out=xtwt[:, q:q + qn],
                                      in_=xt[w, :, q:q + qn])
                    q += qn
                brw = brpool.tile([128, cols], F32)
                nc.sync.dma_start(out=brw[:], in_=br[w])
                wstate[w] = dict(brw=brw, xtwt=xtwt)

            def load_pass2(w):
                # xn (chunked): feeds seg of window w (runs during iter w+1)
                if w >= WPC:
                    return
                xnwt = xnpool.tile([128, XNW], BF16)
                cq = XNW // 2
                for q in range(2):
                    nc.sync.dma_start(out=xnwt[:, q * cq:(q + 1) * cq],
                                      in_=xn[w, :, q * cq:(q + 1) * cq])
                wstate[w]["xnwt"] = xnwt

            def emit_mm1_tanh(w, g):
                ws = wstate[w]
                ph = phpool.tile([128, 2, GRP], F32)
                xt3 = ws["xtwt"][:, g]
                for m in range(2):
                    nc.tensor.matmul(ph[:, m, :], cw[:, m], xt3,
                                     start=True, stop=True,
                                     perf_mode=mybir.MatmulPerfMode.DoubleRow)
                ht = htpool.tile([128, 2, GRP], BF16)
                if zero_bias:
                    nc.scalar.activation(ht[:, :, :], ph[:, :, :],
                                         mybir.ActivationFunctionType.Tanh,
                                         bias=0.0, scale=1.0)
                else:
                    for m in range(2):
                        nc.scalar.activation(ht[:, m, :], ph[:, m, :],
                                             mybir.ActivationFunctionType.Tanh,
                                             bias=cft[:, m:m + 1], scale=1.0)
                ws.setdefault("ht", {})[g] = ht

            def emit_mm2(w, g):
                # plw shares a PSUM bank with pseg: exactly ONE start=True
                # (the bank's first matmul) pending-zeroes the whole 2KB
                # bank; every later matmul relies on per-element has_written
                # (first write overwrites, second accumulates).
                ws = wstate[w]
                if g == 0:
                    pcomb = pspool.tile([128, 324], F32, name="pcomb")
                    ws["pcomb"] = pcomb
                    ws["plw"] = pcomb[:, 260:324]
                    ws["pseg"] = pcomb[:, 0:257]
                ht = ws["ht"].pop(g)
                plw = ws["plw"]
                for t in range(4):
                    c = g * 4 + t
                    for k in range(2):
                        nc.tensor.matmul(plw[:, c:c + 1],
                                         ht[:, k, 128 * t:128 * (t + 1)],
                                         cb[:, C_W2 + k:C_W2 + k + 1],
                                         start=(g == 0 and t == 0 and k == 0),
                                         stop=False, skip_group_check=True)

            NQ = 4  # exp slices per window

            def emit_exp(w, q):
                # exp on quarter-window q's logit columns
                ws = wstate[w]
                qc = cols // NQ
                et = etpool.tile([128, qc], F32)
                ebias = 0.0 if zero_bias else cft[:, 2:3]
                nc.scalar.activation(et[:], ws["plw"][:, q * qc:(q + 1) * qc],
                                     mybir.ActivationFunctionType.Exp,
                                     bias=ebias, scale=1.0)
                ws.setdefault("et", {})[q] = et

            def emit_sbuild(w, g):
                ws = wstate[w]
                qc = cols // NQ
                q = (g * 4) // qc
                et = ws["et"][q]
                sts = []
                for t in range(4):
                    c = g * 4 + t
                    st = spool.tile([128, 128], BF16)
                    nc.vector.tensor_scalar(st[:], iota,
                                            ws["brw"][:, c:c + 1],
                                            et[:, c - q * qc:c - q * qc + 1],
                                            op0=mybir.AluOpType.is_equal,
                                            op1=mybir.AluOpType.mult)
                    sts.append(st)
                ws.setdefault("sts", {})[g] = sts

            def emit_seg(w, g):
                ws = wstate[w]
                sts = ws["sts"].pop(g)
                for t in range(4):
                    c = g * 4 + t
                    nc.tensor.matmul(ws["pseg"][:],
                                     sts[t][:],
                                     ws["xnwt"][:, c * ROW:c * ROW + 257],
                                     start=False,
                                     stop=(g == ng - 1 and t == 3),
                                     skip_group_check=True)
                if g == ng - 1:
                    finalize_window(w)

            def finalize_window(w):
                ws = wstate.pop(w)
                pseg = ws["pseg"]
                dtmp = owpool.tile([128, 1], F32)
                nc.vector.tensor_scalar_add(dtmp[:], pseg[:, 256:257], EPS)
                rec = owpool.tile([128, 1], F32)
                nc.vector.reciprocal(rec[:], dtmp[:])
                ow = owpool.tile([128, HIDDEN], F32)
                nc.vector.tensor_scalar(ow[:], pseg[:, 0:256], rec[:], None,
                                        op0=mybir.AluOpType.mult)
                nc.sync.dma_start(out=out[w * WG:(w + 1) * WG, :], in_=ow[:])

            # ACT warmup: force the exp/tanh table load at t=0 so the first
            # real tanh doesn't pay the ~2.7us ACT_TABLE_LOAD.
            wz = owpool.tile([128, 1], F32)
            nc.vector.memset(wz[:], 0.0)
            wu = owpool.tile([128, 1], F32)
            nc.scalar.activation(wu[:], wz[:],
                                 mybir.ActivationFunctionType.Exp,
                                 bias=0.0, scale=1.0)
            # PE warmup: keep the PE busy while the first window loads so
            # the HAM clock-gate reaches 8/8 before real compute begins.
            dmy = owpool.tile([128, 257], BF16)
            nc.vector.memset(dmy[:], 0.0)
            pdmy = pspool.tile([128, 324], F32, name="pcomb")
            for _ in range(24):
                nc.tensor.matmul(pdmy[:, 0:257], dmy[:, 0:128], dmy[:, 0:257],
                                 start=True, stop=True)

            # main pipeline over flat (window, group) steps; seg lags
            # mm1/mm2 by half a window (exp runs per quarter-window) and
            # S-builds lead their seg by LEAD steps so DVE is never on the
            # PE's critical path.
            steps = [(w, g) for w in range(WPC) for g in range(ng)]
            LAG = 10
            LEAD = 6
            n = len(steps)
            load_pass1(0, first=True)
            load_pass1(1)
            load_pass2(0)
            for i in range(n + LAG):
                if i < n:
                    w, g = steps[i]
                    emit_mm1_tanh(w, g)
                    emit_mm2(w, g)
                    if (g + 1) % (ng // NQ) == 0:
                        emit_exp(w, (g * NQ) // ng)
                j = i - (LAG - LEAD)
                if 0 <= j < n:
                    emit_sbuild(*steps[j])
                if i >= LAG:
                    emit_seg(*steps[i - LAG])
                if i < n:
                    w, g = steps[i]
                    if g == 2:
                        load_pass1(w + 2)
                    elif g == 6:
                        load_pass2(w + 1)
    nc.compile()
    return nc


def _prep_inputs(x, batch, W1, b1, W2, b2):
    batch = np.asarray(batch).astype(np.int64)
    x = np.asarray(x, dtype=np.float32)

    bnds = np.searchsorted(batch, np.arange(0, N_GRAPHS + 1, WG))
    sizes = np.diff(bnds)
    T = int(max(512, ((int(sizes.max()) + GRP - 1) // GRP) * GRP))
    ng = T // GRP
    cols = T // 128

    xbf = x.astype(BF)
    x8 = x.astype(F8)
    batf = batch.astype(np.float32)

    W1 = np.asarray(W1, np.float32)
    cbf = np.zeros((128, CBW), dtype=BF)
    for k in range(2):
        cbf[:, C_W2 + k] = np.asarray(W2, np.float32)[128 * k:128 * (k + 1), 0].astype(BF)
    cbf[:, C_IOTA:C_IOTA + 128] = np.tile(
        np.arange(128, dtype=np.float32), (128, 1)).astype(BF)

    # W1 DoubleRow: cw1[p, mb, i, m] = W1[p + 128*i, 128*mb + m]
    cw1 = np.zeros((128, 2, 2, 128), dtype=F8)
    for mb in range(2):
        for i in range(2):
            cw1[:, mb, i, :] = \
                W1[128 * i:128 * (i + 1), 128 * mb:128 * (mb + 1)].astype(F8)

    cf = np.zeros((128, 131), dtype=np.float32)
    cf[:, 3:131] = np.tile(np.arange(128, dtype=np.float32), (128, 1))
    cf[:, 0] = np.asarray(b1, np.float32)[0:128]
    cf[:, 1] = np.asarray(b1, np.float32)[128:256]
    cf[:, 2] = float(np.asarray(b2, np.float32).reshape(-1)[0])
    zero_bias = bool((np.asarray(b1) == 0).all() and (np.asarray(b2) == 0).all())

    in_maps = []
    for c in range(CORES):
        xn_c = np.zeros((WPC, T, ROW), dtype=BF)
        xt_lin = np.zeros((WPC, 2, 128, T), dtype=F8)
        br_c = np.full((WPC, 128, cols), -1.0, dtype=np.float32)
        for w in range(WPC):
            j = c * WPC + w
            s, e = int(bnds[j]), int(bnds[j + 1])
            sz = e - s
            if sz:
                xn_c[w, :sz, 0:256] = xbf[s:e]
                xn_c[w, :sz, 256] = BF(1.0)
                xt_lin[w, 0, :, :sz] = x8[s:e, 0:128].T
                xt_lin[w, 1, :, :sz] = x8[s:e, 128:256].T
                tmp = np.full(T, -1.0, dtype=np.float32)
                tmp[:sz] = batf[s:e] - (c * 1024 + w * WG)
                br_c[w] = tmp.reshape(cols, 128).T
        # xn swizzle: [w, g*512+t*128+p, d] -> [w, p, (g*4+t)*ROW + d]
        xn_sw = np.ascontiguousarray(
            xn_c.reshape(WPC, ng, 4, 128, ROW).transpose(0, 3, 1, 2, 4)
        ).reshape(WPC, 128, ng * 4 * ROW)
        # xt swizzle: [w, c2, p, g*512+n] -> [w, p, g, c2, n]
        xt_sw = np.ascontiguousarray(
            xt_lin.reshape(WPC, 2, 128, ng, 512).transpose(0, 2, 3, 1, 4))
        in_maps.append(dict(xn=xn_sw, xt=xt_sw, br=br_c, cbf=cbf, cw1=cw1, cf=cf))
    return T, in_maps, zero_bias


_PROGRAM_CACHE = {}


def kernel(x, batch, W1, b1, W2, b2):
    T, in_maps, zb = _prep_inputs(x, batch, W1, b1, W2, b2)
    key = (T, zb)
    if key not in _PROGRAM_CACHE:
        _PROGRAM_CACHE[key] = _build_program(T, zero_bias=zb)
    nc = _PROGRAM_CACHE[key]
    res = run_bass_kernel_spmd(nc, in_maps, list(range(CORES))).results
    return np.concatenate([res[c]["out"] for c in range(CORES)], axis=0)
